# revision 21
# baseline (speedup 1.0000x reference)
"""Conv2d 3x3 VALID via 1D Winograd F(6,3) along H, batch-sharded on 8 cores.

Problem: input [32,128,64,64] f32, weights [256,128,3,3] f32 ->
output [32,256,62,62] f32 (stride 1, no padding).

Scheme (fp16 end to end on device; error ~3e-3 vs f32 reference):
  - Host: Cook-Toom F(6,3) input transform along H with points
    (0,1,-1,2,-2,1/2,-1/2,inf): V[k=0..7, t=0..9, x] per (b, ci), plus an
    F(2,3) tail pair for output rows 60-61. Weight transform G w per
    (k, kw, Cout-half), shipped pre-transposed as lhsT [ci, co].
  - Device (per core, 4 images): M[k][co, t, x] = sum_kw U[k,kw]^T V[k, t, x+kw]
    PSUM-accumulated over kw (f32), 310-col matmuls (5 H-tiles per block).
    Each LDWEIGHTS is shared by the 4 images' matmuls. PSUM is evacuated
    as fp16 by DVE/ACT copies into a staging tile, DMA'd to DRAM.
  - Host: inverse transform Y = A^T M (tiny 6x8 combine) in f32.
"""

import numpy as np

import concourse.bass as bass
import concourse.mybir as mybir
import concourse.tile as tile
from concourse import bacc
from concourse.bass_utils import run_bass_kernel_spmd

F32 = mybir.dt.float32
FP16 = mybir.dt.float16

B, CIN, H, W = 32, 128, 64, 64
COUT, KH, KW = 256, 3, 3
OH, OW = H - KH + 1, W - KW + 1  # 62, 62
N_CORES = 8
BL = B // N_CORES  # 4 images per core

M_TILE = 6          # F(6,3): 6 output rows per tile
NK = M_TILE + 2     # 8 winograd components
NT = 60 // M_TILE   # 10 H-tiles (output rows 0..59)
BLK_T = [(0, 5), (5, 10)]  # 310-col blocks: best matmul efficiency
NBLK = len(BLK_T)
BS = [(t1 - t0) * OW for t0, t1 in BLK_T]  # matmul columns per block
NKT = 4             # F(2,3) tail components (output rows 60-61)
NKM = NK - 2        # comps staged per-image (k6:7 staged in a merged tile)
MOFF = [sum(NKM * s for s in BS[:i]) for i in range(NBLK + 1)]
MLOFF = [sum(BL * 2 * s for s in BS[:i]) for i in range(NBLK + 1)]
# V is shipped block-major flat: [NK, ntl, W] contiguous per block
VOFF = [sum(NK * (t1 - t0) * W for t0, t1 in BLK_T[:i]) for i in range(NBLK + 1)]

# ---------------------------------------------------------------------------
# Cook-Toom transform matrices
# ---------------------------------------------------------------------------


def _derive(m, points):
    """F(m,3) Cook-Toom matrices for given finite points (+infinity).
    Returns At [m,n], G [n,3], Bt [n,n] (f64), n = m+2."""
    from fractions import Fraction

    r = 3
    n = m + r - 1
    pts = [Fraction(p) for p in points]
    At = [[float(a**j) for a in pts] + ([1.0] if j == m - 1 else [0.0])
          for j in range(m)]
    G = []
    for i, a in enumerate(pts):
        N = Fraction(1)
        for j, b in enumerate(pts):
            if i != j:
                N *= a - b
        G.append([float((a**s) / N) for s in range(r)])
    G.append([0.0] * (r - 1) + [1.0])
    A = np.array(At)
    Gf = np.array(G)
    Mm = np.zeros((r * m, n))
    for s in range(r):
        for j in range(m):
            Mm[s * m + j] = A[j] * Gf[:, s]
    Bt = np.zeros((n, n))
    for t in range(n):
        rhs = np.array(
            [1.0 if (t - s) == j else 0.0 for s in range(r) for j in range(m)]
        )
        sol, *_ = np.linalg.lstsq(Mm, rhs, rcond=None)
        assert np.abs(Mm @ sol - rhs).max() < 1e-9
        Bt[:, t] = sol
    return A, Gf, Bt


_PTS6 = [0, 1, -1, 2, -2, 0.5, -0.5]
A6, G6, B6 = _derive(M_TILE, _PTS6)
A2 = np.array([[1.0, 1, 1, 0], [0, 1, -1, -1]])
G2 = np.array([[1.0, 0, 0], [0.5, 0.5, 0.5], [0.5, -0.5, 0.5], [0, 0, 1]])
B2 = np.array([
    [1.0, 0, -1, 0],
    [0, 1, 1, 0],
    [0, -1, 1, 0],
    [0, 1, 0, -1],
])  # Bt[k, r]: V_k = sum_r Bt[k,r] d_r


# ---------------------------------------------------------------------------
# Device kernel
# ---------------------------------------------------------------------------


def _conv_body(nc, tc, m_d, ml_d, mt_d, v_d, vt_d, w_d, wt_d):
    with (
        tc.tile_pool(name="vin", bufs=1) as v_pool,
        tc.tile_pool(name="win", bufs=1) as w_pool,
        tc.tile_pool(name="psum", bufs=8, space=bass.MemorySpace.PSUM) as ps_pool,
        tc.tile_pool(name="stage", bufs=8) as st_pool,
        tc.tile_pool(name="lstage", bufs=2) as lst_pool,
        tc.tile_pool(name="tstage", bufs=2) as tst_pool,
    ):
        w_sb = w_pool.tile([128, 2, NK, KW, 128], FP16, name="w_sb")
        wt_sb = w_pool.tile([128, 2, NKT, KW, 128], FP16, name="wt_sb")
        v_sb = v_pool.tile([128, BL * VOFF[-1]], FP16, name="v_sb")
        vt_sb = v_pool.tile([128, BL, NKT, W], FP16, name="vt_sb")

        # Warm up the PE HAM clock gate during the initial DMA wait: dummy
        # matmuls on the first weight chunk (waits only for the first inbound
        # DMA) so the real stream starts at full clock.
        warm_src = w_sb[:, 0, 0, 0, :]
        ps_warm = ps_pool.tile([128, 512], F32, tag="ps", name="ps")
        for _ in range(16):
            nc.tensor.matmul(ps_warm[:, :128], warm_src, warm_src,
                             start=True, stop=True)

        # Startup: k-major chunks so comp k's data for ALL images lands
        # before comp k+2's, spread across the three DMA-capable queues.
        # ALL inbound DMAs ride the sync queue: within one queue the SDMA
        # transfers are FIFO, so data arrives in k-major order and the first
        # matmul phases start as soon as their chunks land. All 4 images are
        # interleaved inside each chunk (5KB contiguous per partition).
        NTL0 = BLK_T[0][1]
        C0 = BL * NTL0 * W  # per-comp elems in block 0 (all images)
        for k0 in range(0, NK, 2):
            sl = slice(k0 * C0, (k0 + 2) * C0)
            nc.sync.dma_start(
                out=w_sb[:, 0, k0 : k0 + 2], in_=w_d[:, 0, k0 : k0 + 2]
            )
            nc.sync.dma_start(out=v_sb[:, sl], in_=v_d[:, sl])
        # remainder: weights h=1, block 1, tails
        rs = slice(BL * VOFF[1], BL * VOFF[-1])
        nc.sync.dma_start(out=w_sb[:, 1], in_=w_d[:, 1])
        nc.sync.dma_start(out=v_sb[:, rs], in_=v_d[:, rs])
        nc.sync.dma_start(out=wt_sb, in_=wt_d)
        nc.sync.dma_start(out=vt_sb, in_=vt_d)

        def evac(idx, dst, src):
            # alternate PSUM->SBUF fp16 copies between DVE and ACT
            if idx % 2 == 0:
                nc.vector.tensor_copy(dst, src)
            else:
                nc.scalar.activation(dst, src,
                                     mybir.ActivationFunctionType.Copy)

        out_q = [nc.gpsimd, nc.scalar, nc.gpsimd, nc.scalar]

        # (blk, h) order: block-0 V feeds both Cout halves before block-1's
        # data is needed. Comps 0..5 stage per-image and leave as soon as
        # comp 5 is evacuated; comps 6..7 of all images stage in one merged
        # tile with a single DMA, so the per-block trailing work is 2 DMAs.
        def main_block(blk, h):
            t0, t1 = BLK_T[blk]
            S = BS[blk]
            sts = [st_pool.tile([128, NKM, S], FP16, tag="st", name=f"st{b}") for b in range(BL)]
            lst = lst_pool.tile([128, BL, 2, S], FP16, tag="lst", name="lst")
            pss = {}
            for k in range(NK):
                for kw in range(KW):
                    lhsT = w_sb[:, h, k, kw, :]
                    for b in range(BL):
                        if kw == 0:
                            pss[b] = ps_pool.tile([128, 512], F32, tag="ps", name=f"ps{b}")
                        ntl = t1 - t0
                        voff = BL * VOFF[blk] + (k * BL + b) * ntl * W
                        rhs = v_sb[:, voff : voff + ntl * W].rearrange(
                            "p (t x) -> p t x", x=W
                        )[:, :, kw : kw + OW]
                        nc.tensor.matmul(
                            pss[b][:, :S].rearrange("p (t x) -> p t x", x=OW),
                            lhsT,
                            rhs,
                            start=(kw == 0),
                            stop=(kw == KW - 1),
                        )
                for b in range(BL):
                    if k < NKM:
                        evac(k * BL + b, sts[b][:, k, :], pss[b][:, :S])
                    else:
                        evac(k * BL + b, lst[:, b, k - NKM, :], pss[b][:, :S])
                if k == NKM - 1:
                    for b in range(BL):
                        out_q[b].dma_start(
                            out=m_d[h, :, b, MOFF[blk] : MOFF[blk + 1]],
                            in_=sts[b].rearrange("p k s -> p (k s)"),
                        )
            nc.gpsimd.dma_start(
                out=ml_d[h, :, MLOFF[blk] : MLOFF[blk + 1]],
                in_=lst.rearrange("p b k s -> p (b k s)"),
            )

        def tail_block(h):
            # F(2,3) tail: output rows 60-61
            tst = tst_pool.tile([128, BL, NKT, OW], FP16, tag="tst", name="tst")
            tps = {}
            for k in range(NKT):
                for kw in range(KW):
                    lhsT = wt_sb[:, h, k, kw, :]
                    for b in range(BL):
                        if kw == 0:
                            tps[b] = ps_pool.tile([128, 512], F32, tag="ps", name=f"tps{b}")
                        nc.tensor.matmul(
                            tps[b][:, :OW],
                            lhsT,
                            vt_sb[:, b, k, kw : kw + OW],
                            start=(kw == 0),
                            stop=(kw == KW - 1),
                        )
                for b in range(BL):
                    evac(k * BL + b, tst[:, b, k, :], tps[b][:, :OW])
            nc.scalar.dma_start(
                out=mt_d[h], in_=tst.rearrange("p b k s -> p (b k s)")
            )

        # tails run before the final block so the run ends on a block whose
        # outbound data has been streaming since its comp-5 evacuations
        main_block(0, 0)
        main_block(0, 1)
        main_block(1, 0)
        tail_block(0)
        tail_block(1)
        main_block(1, 1)


def build_module():
    nc = bacc.Bacc(
        "TRN2", target_bir_lowering=False, debug=False, num_devices=N_CORES
    )
    v_d = nc.dram_tensor(
        "v_in", [CIN, BL * VOFF[-1]], FP16, kind="ExternalInput"
    ).ap()
    vt_d = nc.dram_tensor(
        "vt_in", [CIN, BL, NKT, W], FP16, kind="ExternalInput"
    ).ap()
    w_d = nc.dram_tensor(
        "w_t", [CIN, 2, NK, KW, 128], FP16, kind="ExternalInput"
    ).ap()
    wt_d = nc.dram_tensor(
        "wt_t", [CIN, 2, NKT, KW, 128], FP16, kind="ExternalInput"
    ).ap()
    m_d = nc.dram_tensor(
        "m_out", [2, 128, BL, MOFF[-1]], FP16, kind="ExternalOutput"
    ).ap()
    ml_d = nc.dram_tensor(
        "ml_out", [2, 128, MLOFF[-1]], FP16, kind="ExternalOutput"
    ).ap()
    mt_d = nc.dram_tensor(
        "mt_out", [2, 128, BL * NKT * OW], FP16, kind="ExternalOutput"
    ).ap()
    with tile.TileContext(nc) as tc:
        _conv_body(nc, tc, m_d, ml_d, mt_d, v_d, vt_d, w_d, wt_d)
    nc.compile()
    return nc


_NC_CACHE = {}


def _get_module():
    if "nc" not in _NC_CACHE:
        _NC_CACHE["nc"] = build_module()
    return _NC_CACHE["nc"]


# ---------------------------------------------------------------------------
# Host transforms
# ---------------------------------------------------------------------------


def _host_transforms(input_image: np.ndarray, weights: np.ndarray):
    x = input_image.astype(np.float32)
    win = np.lib.stride_tricks.sliding_window_view(x, NK, axis=2)[:, :, ::M_TILE]
    win = win[:, :, :NT]  # [B, C, NT, W, 8]
    B6f = B6.astype(np.float32)
    V = np.einsum("kr,bctwr->bcktw", B6f, win, optimize=True)  # [B,C,NK,NT,W]
    # per core: [C, blk-major (k, b, t, w)] with the core's 4 images b
    Vc = []
    for i in range(N_CORES):
        Vi = V[i * BL : (i + 1) * BL]  # [BL, C, NK, NT, W]
        parts = [
            Vi[:, :, :, t0:t1].transpose(1, 2, 0, 3, 4).reshape(CIN, -1)
            for t0, t1 in BLK_T
        ]
        Vc.append(np.ascontiguousarray(
            np.concatenate(parts, axis=1), dtype=np.float16))
    V = Vc

    d = x[:, :, 60:64]  # [B, C, 4, W]
    Vt = np.einsum("kr,bcrw->bckw", B2.astype(np.float32), d, optimize=True)
    Vt = Vt.astype(np.float16)

    wf = weights.astype(np.float32)  # [co, ci, kh, kw]
    U = np.einsum("kr,ocrw->cwko", G6.astype(np.float32), wf, optimize=True)
    U = U.reshape(CIN, KW, NK, 2, 128).transpose(0, 3, 2, 1, 4)
    U = np.ascontiguousarray(U, dtype=np.float16)
    Ut = np.einsum("kr,ocrw->cwko", G2.astype(np.float32), wf, optimize=True)
    Ut = Ut.reshape(CIN, KW, NKT, 2, 128).transpose(0, 3, 2, 1, 4)
    Ut = np.ascontiguousarray(Ut, dtype=np.float16)
    return V, Vt, U, Ut


def _host_combine(m_list, ml_list, mt_list):
    """m: [2,128,BL,NBLK,NKM*S]; ml: [2,NBLK,128,BL*2*S]; mt: [2,128,BL*NKT*OW].
    Returns [B, COUT, OH, OW] f32."""
    out = np.empty((B, COUT, OH, OW), np.float32)
    A6f = A6.astype(np.float32)
    A2f = A2.astype(np.float32)
    for i, (m, ml, mt) in enumerate(zip(m_list, ml_list, mt_list)):
        mm = np.empty((2, 128, BL, NK, NT, OW), np.float32)
        mf = m.astype(np.float32)
        mlf = ml.astype(np.float32)
        for blk, (t0, t1) in enumerate(BLK_T):
            ntl = t1 - t0
            seg = mf[..., MOFF[blk] : MOFF[blk + 1]]
            mm[:, :, :, :NKM, t0:t1] = seg.reshape(2, 128, BL, NKM, ntl, OW)
            segl = mlf[..., MLOFF[blk] : MLOFF[blk + 1]]
            mm[:, :, :, NKM:, t0:t1] = segl.reshape(
                2, 128, BL, 2, ntl, OW
            )
        y = np.einsum("jk,hobktx->bhotjx", A6f, mm, optimize=True)
        y = y.reshape(BL, COUT, NT * M_TILE, OW)
        sl = out[i * BL : (i + 1) * BL]
        sl[:, :, :60] = y
        mtf = mt.astype(np.float32).reshape(2, 128, BL, NKT, OW)
        yt = np.einsum("jk,hobkx->bhojx", A2f, mtf, optimize=True)
        sl[:, :, 60:62] = yt.reshape(BL, COUT, 2, OW)
    return out


def make_in_maps(input_image: np.ndarray, weights: np.ndarray):
    V, Vt, U, Ut = _host_transforms(
        np.ascontiguousarray(input_image, dtype=np.float32),
        np.ascontiguousarray(weights, dtype=np.float32),
    )
    return [
        {
            "v_in": V[i],
            "vt_in": np.ascontiguousarray(
                Vt[i * BL : (i + 1) * BL].transpose(1, 0, 2, 3)
            ),
            "w_t": U,
            "wt_t": Ut,
        }
        for i in range(N_CORES)
    ]


def kernel(input_image: np.ndarray, weights: np.ndarray) -> np.ndarray:
    in_maps = make_in_maps(input_image, weights)
    nc = _get_module()
    res = run_bass_kernel_spmd(nc, in_maps, list(range(N_CORES))).results
    return _host_combine(
        [r["m_out"] for r in res],
        [r["ml_out"] for r in res],
        [r["mt_out"] for r in res],
    )


# revision 22
# speedup vs baseline: 1.0194x; 1.0194x over previous
"""Conv2d 3x3 VALID via 1D Winograd F(6,3) along H, batch-sharded on 8 cores.

Problem: input [32,128,64,64] f32, weights [256,128,3,3] f32 ->
output [32,256,62,62] f32 (stride 1, no padding).

Scheme (fp16 end to end on device; error ~3e-3 vs f32 reference):
  - Host: Cook-Toom F(6,3) input transform along H with points
    (0,1,-1,2,-2,1/2,-1/2,inf): V[k=0..7, t=0..9, x] per (b, ci), plus an
    F(2,3) tail pair for output rows 60-61. Weight transform G w per
    (k, kw, Cout-half), shipped pre-transposed as lhsT [ci, co].
  - Device (per core, 4 images): M[k][co, t, x] = sum_kw U[k,kw]^T V[k, t, x+kw]
    PSUM-accumulated over kw (f32), 310-col matmuls (5 H-tiles per block).
    Each LDWEIGHTS is shared by the 4 images' matmuls. PSUM is evacuated
    as fp16 by DVE/ACT copies into a staging tile, DMA'd to DRAM.
  - Host: inverse transform Y = A^T M (tiny 6x8 combine) in f32.
"""

import numpy as np

import concourse.bass as bass
import concourse.mybir as mybir
import concourse.tile as tile
from concourse import bacc
from concourse.bass_utils import run_bass_kernel_spmd

F32 = mybir.dt.float32
FP16 = mybir.dt.float16

B, CIN, H, W = 32, 128, 64, 64
COUT, KH, KW = 256, 3, 3
OH, OW = H - KH + 1, W - KW + 1  # 62, 62
N_CORES = 8
BL = B // N_CORES  # 4 images per core

M_TILE = 6          # F(6,3): 6 output rows per tile
NK = M_TILE + 2     # 8 winograd components
NT = 60 // M_TILE   # 10 H-tiles (output rows 0..59)
BLK_T = [(0, 4), (4, 10)]  # smaller first block: startup slack
NBLK = len(BLK_T)
BS = [(t1 - t0) * OW for t0, t1 in BLK_T]  # matmul columns per block
NKT = 4             # F(2,3) tail components (output rows 60-61)
NKM = NK - 2        # comps staged per-image (k6:7 staged in a merged tile)
MOFF = [sum(NKM * s for s in BS[:i]) for i in range(NBLK + 1)]
MLOFF = [sum(BL * 2 * s for s in BS[:i]) for i in range(NBLK + 1)]
# V is shipped block-major flat: [NK, ntl, W] contiguous per block
VOFF = [sum(NK * (t1 - t0) * W for t0, t1 in BLK_T[:i]) for i in range(NBLK + 1)]

# ---------------------------------------------------------------------------
# Cook-Toom transform matrices
# ---------------------------------------------------------------------------


def _derive(m, points):
    """F(m,3) Cook-Toom matrices for given finite points (+infinity).
    Returns At [m,n], G [n,3], Bt [n,n] (f64), n = m+2."""
    from fractions import Fraction

    r = 3
    n = m + r - 1
    pts = [Fraction(p) for p in points]
    At = [[float(a**j) for a in pts] + ([1.0] if j == m - 1 else [0.0])
          for j in range(m)]
    G = []
    for i, a in enumerate(pts):
        N = Fraction(1)
        for j, b in enumerate(pts):
            if i != j:
                N *= a - b
        G.append([float((a**s) / N) for s in range(r)])
    G.append([0.0] * (r - 1) + [1.0])
    A = np.array(At)
    Gf = np.array(G)
    Mm = np.zeros((r * m, n))
    for s in range(r):
        for j in range(m):
            Mm[s * m + j] = A[j] * Gf[:, s]
    Bt = np.zeros((n, n))
    for t in range(n):
        rhs = np.array(
            [1.0 if (t - s) == j else 0.0 for s in range(r) for j in range(m)]
        )
        sol, *_ = np.linalg.lstsq(Mm, rhs, rcond=None)
        assert np.abs(Mm @ sol - rhs).max() < 1e-9
        Bt[:, t] = sol
    return A, Gf, Bt


_PTS6 = [0, 1, -1, 2, -2, 0.5, -0.5]
A6, G6, B6 = _derive(M_TILE, _PTS6)
A2 = np.array([[1.0, 1, 1, 0], [0, 1, -1, -1]])
G2 = np.array([[1.0, 0, 0], [0.5, 0.5, 0.5], [0.5, -0.5, 0.5], [0, 0, 1]])
B2 = np.array([
    [1.0, 0, -1, 0],
    [0, 1, 1, 0],
    [0, -1, 1, 0],
    [0, 1, 0, -1],
])  # Bt[k, r]: V_k = sum_r Bt[k,r] d_r


# ---------------------------------------------------------------------------
# Device kernel
# ---------------------------------------------------------------------------


def _conv_body(nc, tc, m_d, ml_d, mt_d, v_d, vt_d, w_d, wt_d):
    with (
        tc.tile_pool(name="vin", bufs=1) as v_pool,
        tc.tile_pool(name="win", bufs=1) as w_pool,
        tc.tile_pool(name="psum", bufs=8, space=bass.MemorySpace.PSUM) as ps_pool,
        tc.tile_pool(name="stage", bufs=8) as st_pool,
        tc.tile_pool(name="lstage", bufs=2) as lst_pool,
        tc.tile_pool(name="tstage", bufs=2) as tst_pool,
    ):
        w_sb = w_pool.tile([128, 2, NK, KW, 128], FP16, name="w_sb")
        wt_sb = w_pool.tile([128, 2, NKT, KW, 128], FP16, name="wt_sb")
        v_sb = v_pool.tile([128, BL * VOFF[-1]], FP16, name="v_sb")
        vt_sb = v_pool.tile([128, BL, NKT, W], FP16, name="vt_sb")

        # Warm up the PE HAM clock gate during the initial DMA wait: dummy
        # matmuls on the first weight chunk (waits only for the first inbound
        # DMA) so the real stream starts at full clock.
        warm_src = w_sb[:, 0, 0, 0, :]
        ps_warm = ps_pool.tile([128, 512], F32, tag="ps", name="ps")
        for _ in range(16):
            nc.tensor.matmul(ps_warm[:, :128], warm_src, warm_src,
                             start=True, stop=True)

        # Startup: k-major chunks so comp k's data for ALL images lands
        # before comp k+2's, spread across the three DMA-capable queues.
        # ALL inbound DMAs ride the sync queue: within one queue the SDMA
        # transfers are FIFO, so data arrives in k-major order and the first
        # matmul phases start as soon as their chunks land. All 4 images are
        # interleaved inside each chunk (5KB contiguous per partition).
        NTL0 = BLK_T[0][1]
        C0 = BL * NTL0 * W  # per-comp elems in block 0 (all images)
        for k0 in range(0, NK, 2):
            sl = slice(k0 * C0, (k0 + 2) * C0)
            nc.sync.dma_start(
                out=w_sb[:, 0, k0 : k0 + 2], in_=w_d[:, 0, k0 : k0 + 2]
            )
            nc.sync.dma_start(out=v_sb[:, sl], in_=v_d[:, sl])
        # remainder: weights h=1, block 1, tails
        rs = slice(BL * VOFF[1], BL * VOFF[-1])
        nc.sync.dma_start(out=w_sb[:, 1], in_=w_d[:, 1])
        nc.sync.dma_start(out=v_sb[:, rs], in_=v_d[:, rs])
        nc.sync.dma_start(out=wt_sb, in_=wt_d)
        nc.sync.dma_start(out=vt_sb, in_=vt_d)

        def evac(idx, dst, src):
            # alternate PSUM->SBUF fp16 copies between DVE and ACT
            if idx % 2 == 0:
                nc.vector.tensor_copy(dst, src)
            else:
                nc.scalar.activation(dst, src,
                                     mybir.ActivationFunctionType.Copy)

        out_q = [nc.gpsimd, nc.scalar, nc.gpsimd, nc.scalar]

        # (blk, h) order: block-0 V feeds both Cout halves before block-1's
        # data is needed. Comps 0..5 stage per-image and leave as soon as
        # comp 5 is evacuated; comps 6..7 of all images stage in one merged
        # tile with a single DMA, so the per-block trailing work is 2 DMAs.
        def main_block(blk, h):
            t0, t1 = BLK_T[blk]
            S = BS[blk]
            sts = [st_pool.tile([128, NKM, S], FP16, tag="st", name=f"st{b}") for b in range(BL)]
            lst = lst_pool.tile([128, BL, 2, S], FP16, tag="lst", name="lst")
            pss = {}
            for k in range(NK):
                for kw in range(KW):
                    lhsT = w_sb[:, h, k, kw, :]
                    for b in range(BL):
                        if kw == 0:
                            pss[b] = ps_pool.tile([128, 512], F32, tag="ps", name=f"ps{b}")
                        ntl = t1 - t0
                        voff = BL * VOFF[blk] + (k * BL + b) * ntl * W
                        rhs = v_sb[:, voff : voff + ntl * W].rearrange(
                            "p (t x) -> p t x", x=W
                        )[:, :, kw : kw + OW]
                        nc.tensor.matmul(
                            pss[b][:, :S].rearrange("p (t x) -> p t x", x=OW),
                            lhsT,
                            rhs,
                            start=(kw == 0),
                            stop=(kw == KW - 1),
                        )
                for b in range(BL):
                    if k < NKM:
                        evac(k * BL + b, sts[b][:, k, :], pss[b][:, :S])
                    else:
                        evac(k * BL + b, lst[:, b, k - NKM, :], pss[b][:, :S])
                if k == NKM - 1:
                    for b in range(BL):
                        out_q[b].dma_start(
                            out=m_d[h, :, b, MOFF[blk] : MOFF[blk + 1]],
                            in_=sts[b].rearrange("p k s -> p (k s)"),
                        )
            nc.gpsimd.dma_start(
                out=ml_d[h, :, MLOFF[blk] : MLOFF[blk + 1]],
                in_=lst.rearrange("p b k s -> p (b k s)"),
            )

        def tail_block(h):
            # F(2,3) tail: output rows 60-61
            tst = tst_pool.tile([128, BL, NKT, OW], FP16, tag="tst", name="tst")
            tps = {}
            for k in range(NKT):
                for kw in range(KW):
                    lhsT = wt_sb[:, h, k, kw, :]
                    for b in range(BL):
                        if kw == 0:
                            tps[b] = ps_pool.tile([128, 512], F32, tag="ps", name=f"tps{b}")
                        nc.tensor.matmul(
                            tps[b][:, :OW],
                            lhsT,
                            vt_sb[:, b, k, kw : kw + OW],
                            start=(kw == 0),
                            stop=(kw == KW - 1),
                        )
                for b in range(BL):
                    evac(k * BL + b, tst[:, b, k, :], tps[b][:, :OW])
            nc.scalar.dma_start(
                out=mt_d[h], in_=tst.rearrange("p b k s -> p (b k s)")
            )

        # tails run before the final block so the run ends on a block whose
        # outbound data has been streaming since its comp-5 evacuations
        main_block(0, 0)
        main_block(0, 1)
        main_block(1, 0)
        main_block(1, 1)
        tail_block(0)
        tail_block(1)


def build_module():
    nc = bacc.Bacc(
        "TRN2", target_bir_lowering=False, debug=False, num_devices=N_CORES
    )
    v_d = nc.dram_tensor(
        "v_in", [CIN, BL * VOFF[-1]], FP16, kind="ExternalInput"
    ).ap()
    vt_d = nc.dram_tensor(
        "vt_in", [CIN, BL, NKT, W], FP16, kind="ExternalInput"
    ).ap()
    w_d = nc.dram_tensor(
        "w_t", [CIN, 2, NK, KW, 128], FP16, kind="ExternalInput"
    ).ap()
    wt_d = nc.dram_tensor(
        "wt_t", [CIN, 2, NKT, KW, 128], FP16, kind="ExternalInput"
    ).ap()
    m_d = nc.dram_tensor(
        "m_out", [2, 128, BL, MOFF[-1]], FP16, kind="ExternalOutput"
    ).ap()
    ml_d = nc.dram_tensor(
        "ml_out", [2, 128, MLOFF[-1]], FP16, kind="ExternalOutput"
    ).ap()
    mt_d = nc.dram_tensor(
        "mt_out", [2, 128, BL * NKT * OW], FP16, kind="ExternalOutput"
    ).ap()
    with tile.TileContext(nc) as tc:
        _conv_body(nc, tc, m_d, ml_d, mt_d, v_d, vt_d, w_d, wt_d)
    nc.compile()
    return nc


_NC_CACHE = {}


def _get_module():
    if "nc" not in _NC_CACHE:
        _NC_CACHE["nc"] = build_module()
    return _NC_CACHE["nc"]


# ---------------------------------------------------------------------------
# Host transforms
# ---------------------------------------------------------------------------


def _host_transforms(input_image: np.ndarray, weights: np.ndarray):
    x = input_image.astype(np.float32)
    win = np.lib.stride_tricks.sliding_window_view(x, NK, axis=2)[:, :, ::M_TILE]
    win = win[:, :, :NT]  # [B, C, NT, W, 8]
    B6f = B6.astype(np.float32)
    V = np.einsum("kr,bctwr->bcktw", B6f, win, optimize=True)  # [B,C,NK,NT,W]
    # per core: [C, blk-major (k, b, t, w)] with the core's 4 images b
    Vc = []
    for i in range(N_CORES):
        Vi = V[i * BL : (i + 1) * BL]  # [BL, C, NK, NT, W]
        parts = [
            Vi[:, :, :, t0:t1].transpose(1, 2, 0, 3, 4).reshape(CIN, -1)
            for t0, t1 in BLK_T
        ]
        Vc.append(np.ascontiguousarray(
            np.concatenate(parts, axis=1), dtype=np.float16))
    V = Vc

    d = x[:, :, 60:64]  # [B, C, 4, W]
    Vt = np.einsum("kr,bcrw->bckw", B2.astype(np.float32), d, optimize=True)
    Vt = Vt.astype(np.float16)

    wf = weights.astype(np.float32)  # [co, ci, kh, kw]
    U = np.einsum("kr,ocrw->cwko", G6.astype(np.float32), wf, optimize=True)
    U = U.reshape(CIN, KW, NK, 2, 128).transpose(0, 3, 2, 1, 4)
    U = np.ascontiguousarray(U, dtype=np.float16)
    Ut = np.einsum("kr,ocrw->cwko", G2.astype(np.float32), wf, optimize=True)
    Ut = Ut.reshape(CIN, KW, NKT, 2, 128).transpose(0, 3, 2, 1, 4)
    Ut = np.ascontiguousarray(Ut, dtype=np.float16)
    return V, Vt, U, Ut


def _host_combine(m_list, ml_list, mt_list):
    """m: [2,128,BL,NBLK,NKM*S]; ml: [2,NBLK,128,BL*2*S]; mt: [2,128,BL*NKT*OW].
    Returns [B, COUT, OH, OW] f32."""
    out = np.empty((B, COUT, OH, OW), np.float32)
    A6f = A6.astype(np.float32)
    A2f = A2.astype(np.float32)
    for i, (m, ml, mt) in enumerate(zip(m_list, ml_list, mt_list)):
        mm = np.empty((2, 128, BL, NK, NT, OW), np.float32)
        mf = m.astype(np.float32)
        mlf = ml.astype(np.float32)
        for blk, (t0, t1) in enumerate(BLK_T):
            ntl = t1 - t0
            seg = mf[..., MOFF[blk] : MOFF[blk + 1]]
            mm[:, :, :, :NKM, t0:t1] = seg.reshape(2, 128, BL, NKM, ntl, OW)
            segl = mlf[..., MLOFF[blk] : MLOFF[blk + 1]]
            mm[:, :, :, NKM:, t0:t1] = segl.reshape(
                2, 128, BL, 2, ntl, OW
            )
        y = np.einsum("jk,hobktx->bhotjx", A6f, mm, optimize=True)
        y = y.reshape(BL, COUT, NT * M_TILE, OW)
        sl = out[i * BL : (i + 1) * BL]
        sl[:, :, :60] = y
        mtf = mt.astype(np.float32).reshape(2, 128, BL, NKT, OW)
        yt = np.einsum("jk,hobkx->bhojx", A2f, mtf, optimize=True)
        sl[:, :, 60:62] = yt.reshape(BL, COUT, 2, OW)
    return out


def make_in_maps(input_image: np.ndarray, weights: np.ndarray):
    V, Vt, U, Ut = _host_transforms(
        np.ascontiguousarray(input_image, dtype=np.float32),
        np.ascontiguousarray(weights, dtype=np.float32),
    )
    return [
        {
            "v_in": V[i],
            "vt_in": np.ascontiguousarray(
                Vt[i * BL : (i + 1) * BL].transpose(1, 0, 2, 3)
            ),
            "w_t": U,
            "wt_t": Ut,
        }
        for i in range(N_CORES)
    ]


def kernel(input_image: np.ndarray, weights: np.ndarray) -> np.ndarray:
    in_maps = make_in_maps(input_image, weights)
    nc = _get_module()
    res = run_bass_kernel_spmd(nc, in_maps, list(range(N_CORES))).results
    return _host_combine(
        [r["m_out"] for r in res],
        [r["ml_out"] for r in res],
        [r["mt_out"] for r in res],
    )
